# revision 17
# baseline (speedup 1.0000x reference)
"""GPTQ 4-bit quantized linear (nn_Ex4bitLinear) for 8 Trainium2 NeuronCores.

Computes out = x @ dequant(qweight, qzeros, scales) + bias where
  x:       [8192, 4096] fp32
  qweight: [512, 4096] int32 (8 x 4-bit along K per word)
  qzeros:  [32, 512] int32 (8 x 4-bit along N per word)
  scales:  [32, 4096] fp32, groupsize 128 (standard g_idx = k // 128)
  bias:    [4096] fp32

Sharding: 2-way on tokens x 4-way on out-features (core = ti*4 + ni).
Each core dequantizes its [4096, 1024] weight shard to bf16 in SBUF once,
streams x tiles (cast fp32->bf16 in-DMA, transposed via the DMA xbar) and
runs chunk-accumulated bf16 matmuls with fp32 PSUM.
"""

import numpy as np
from contextlib import ExitStack

import concourse.bass as bass
import concourse.mybir as mybir
import concourse.tile as tile
from concourse import bacc
from concourse.bass_utils import run_bass_kernel_spmd

P = 128
GROUPSIZE = 128

# Full problem dims.
TOKENS_F, K_F, N_F = 8192, 4096, 4096
TSHARD, NSHARD = 2, 4
N_CORES = TSHARD * NSHARD


def build_kernel(T_s=TOKENS_F // TSHARD, K=K_F, N_s=N_F // NSHARD):
    """Build the per-core Bass program. T_s tokens, K contraction, N_s out dims."""
    assert T_s % P == 0 and K % P == 0 and N_s % 512 == 0
    C = K // P                 # contraction chunks == quant groups
    NB = N_s // 512            # 512-wide output column blocks
    TT = T_s // P              # token tiles

    nc = bacc.Bacc("TRN2", target_bir_lowering=False, debug=False)
    x_d = nc.dram_tensor("x", [T_s, K], mybir.dt.float32, kind="ExternalInput")
    qw_d = nc.dram_tensor("qw", [K // 8, N_s], mybir.dt.int32, kind="ExternalInput")
    sc_d = nc.dram_tensor("sc", [C, N_s], mybir.dt.float32, kind="ExternalInput")
    # zs = (unpacked_zero + 1) * scale, precomputed host-side from qzeros
    zs_d = nc.dram_tensor("zs", [C, N_s], mybir.dt.float32, kind="ExternalInput")
    bias_d = nc.dram_tensor("bias", [N_s], mybir.dt.float32, kind="ExternalInput")
    out_d = nc.dram_tensor("out", [T_s, N_s], mybir.dt.float32, kind="ExternalOutput")

    # Constants embedded in the NEFF.
    shiftv_np = (4 * (np.arange(P) % 8)).astype(np.int32).reshape(P, 1)
    shiftv_d = nc.inline_tensor(shiftv_np, name="shiftv")

    with tile.TileContext(nc) as tc, ExitStack() as ctx:
        const = ctx.enter_context(tc.tile_pool(name="const", bufs=1))
        wpool = ctx.enter_context(tc.tile_pool(name="wpool", bufs=C))
        dq = ctx.enter_context(tc.tile_pool(name="dq", bufs=2))
        xp = ctx.enter_context(tc.tile_pool(name="xp", bufs=3))
        op = ctx.enter_context(tc.tile_pool(name="op", bufs=4))
        psum = ctx.enter_context(tc.tile_pool(name="psum", bufs=4, space="PSUM"))

        # ---- constants / setup ----
        # Load once then re-materialize via DVE so downstream DVE consumers
        # sync on same-engine program order instead of extra DMA semaphores
        # (TT/TS instructions have very limited sync-wait slots).
        shiftv0 = const.tile([P, 1], mybir.dt.int32)
        nc.sync.dma_start(shiftv0[:], shiftv_d[:, :])
        shiftv = const.tile([P, 1], mybir.dt.int32)
        nc.vector.tensor_copy(shiftv[:], shiftv0[:])
        bias_rep0 = const.tile([P, N_s], mybir.dt.float32)
        nc.sync.dma_start(bias_rep0[:], bass.AP(bias_d, 0, [[0, P], [1, N_s]]))
        bias_rep = const.tile([P, N_s], mybir.dt.float32)
        nc.vector.tensor_copy(bias_rep[:], bias_rep0[:])

        # ---- dequantize W chunk by chunk into resident bf16 [C][128, N_s] ----
        w_tiles = []
        for c in range(C):
            qb = dq.tile([P, N_s], mybir.dt.int32, tag="qb")
            # partition p = r*8 + j reads packed row 16c + r, all N_s cols
            src = bass.AP(qw_d, c * 16 * N_s, [[N_s, 16], [0, 8], [1, N_s]])
            nc.sync.dma_start(qb[:], src)
            s_rep = dq.tile([P, N_s], mybir.dt.float32, tag="s_rep")
            nc.sync.dma_start(s_rep[:], bass.AP(sc_d, c * N_s, [[0, P], [1, N_s]]))
            zs_rep = dq.tile([P, N_s], mybir.dt.float32, tag="zs_rep")
            nc.sync.dma_start(zs_rep[:], bass.AP(zs_d, c * N_s, [[0, P], [1, N_s]]))

            # qb = (qb >> (4 * (p % 8))) & 0xF ; q4f = float(qb)
            nc.vector.tensor_tensor(
                qb[:], qb[:], shiftv[:].to_broadcast((P, N_s)),
                mybir.AluOpType.logical_shift_right,
            )
            nc.vector.tensor_scalar(
                qb[:], qb[:], 0xF, None, mybir.AluOpType.bitwise_and,
            )
            q4f = dq.tile([P, N_s], mybir.dt.float32, tag="q4f")
            nc.vector.tensor_copy(q4f[:], qb[:])
            # q4f = q4f * s ; W[c] = q4f - zs   (bf16 writeback)
            nc.vector.tensor_tensor(q4f[:], q4f[:], s_rep[:], mybir.AluOpType.mult)
            w = wpool.tile([P, N_s], mybir.dt.bfloat16, tag="w")
            nc.vector.tensor_tensor(w[:], q4f[:], zs_rep[:], mybir.AluOpType.subtract)
            w_tiles.append(w)

        # ---- matmul over token tiles ----
        for t in range(TT):
            x_bf = xp.tile([P, K], mybir.dt.bfloat16, tag="x_bf")
            nc.gpsimd.dma_start(x_bf[:], x_d[t * P:(t + 1) * P, :])  # cast f32->bf16
            xt = xp.tile([P, C, P], mybir.dt.bfloat16, tag="xt")
            nc.sync.dma_start_transpose(xt[:], x_bf[:])

            psums = [psum.tile([P, 512], mybir.dt.float32, tag="ps", name=f"ps{nb}") for nb in range(NB)]
            for c in range(C):
                lhsT = xt[:, c, :]
                for nb in range(NB):
                    nc.tensor.matmul(
                        psums[nb][:], lhsT, w_tiles[c][:, nb * 512:(nb + 1) * 512],
                        start=(c == 0), stop=(c == C - 1),
                    )
            for nb in range(NB):
                o = op.tile([P, 512], mybir.dt.float32, tag="o")
                nc.vector.tensor_tensor(
                    o[:], psums[nb][:], bias_rep[:, nb * 512:(nb + 1) * 512],
                    mybir.AluOpType.add,
                )
                nc.sync.dma_start(
                    out_d[t * P:(t + 1) * P, nb * 512:(nb + 1) * 512], o[:],
                )

    nc.compile()
    return nc


_cache = {}


def _get_kernel(T_s, K, N_s):
    key = (T_s, K, N_s)
    if key not in _cache:
        _cache[key] = build_kernel(T_s, K, N_s)
    return _cache[key]


def make_in_maps(x, qweight, qzeros, scales, bias):
    """Split full inputs into per-core input dicts (2 token x 4 feature shards)."""
    t_sz = x.shape[0] // TSHARD
    n_sz = qweight.shape[1] // NSHARD
    # Unpack the (tiny) packed zero-points and fold the +1 and scale on host:
    # zs[g, n] = (z[g, n] + 1) * scales[g, n]
    shifts = (np.arange(8, dtype=np.int32) * 4)
    z = ((qzeros[:, :, None] >> shifts[None, None, :]) & 0xF).reshape(
        qzeros.shape[0], -1)
    zs = ((z + 1).astype(np.float32) * scales).astype(np.float32)
    in_maps = []
    for core in range(N_CORES):
        ti, ni = divmod(core, NSHARD)
        in_maps.append({
            "x": np.ascontiguousarray(x[ti * t_sz:(ti + 1) * t_sz, :]),
            "qw": np.ascontiguousarray(qweight[:, ni * n_sz:(ni + 1) * n_sz]),
            "sc": np.ascontiguousarray(scales[:, ni * n_sz:(ni + 1) * n_sz]),
            "zs": np.ascontiguousarray(zs[:, ni * n_sz:(ni + 1) * n_sz]),
            "bias": np.ascontiguousarray(bias[ni * n_sz:(ni + 1) * n_sz]),
        })
    return in_maps


def assemble(results, tokens, n):
    t_sz = tokens // TSHARD
    n_sz = n // NSHARD
    out = np.empty((tokens, n), dtype=np.float32)
    for core in range(N_CORES):
        ti, ni = divmod(core, NSHARD)
        out[ti * t_sz:(ti + 1) * t_sz, ni * n_sz:(ni + 1) * n_sz] = results[core]["out"]
    return out


def kernel(x, qweight, qzeros, scales, g_idx, bias, _trace=False):
    x = np.asarray(x, dtype=np.float32)
    qweight = np.asarray(qweight, dtype=np.int32)
    qzeros = np.asarray(qzeros, dtype=np.int32)
    scales = np.asarray(scales, dtype=np.float32)
    bias = np.asarray(bias, dtype=np.float32)

    nc = _get_kernel(x.shape[0] // TSHARD, x.shape[1], qweight.shape[1] // NSHARD)
    in_maps = make_in_maps(x, qweight, qzeros, scales, bias)
    res = run_bass_kernel_spmd(
        nc, in_maps, core_ids=list(range(N_CORES)), trace=_trace,
    )
    out = assemble(res.results, x.shape[0], qweight.shape[1])
    if _trace:
        kernel.last_result = res
    return out


# revision 32
# speedup vs baseline: 6.5192x; 6.5192x over previous
"""GPTQ 4-bit quantized linear (nn_Ex4bitLinear) for 8 Trainium2 NeuronCores.

Computes out = x @ dequant(qweight, qzeros, scales) + bias where
  x:       [8192, 4096] fp32
  qweight: [512, 4096] int32 (8 x 4-bit along K per word)
  qzeros:  [32, 512] int32 (8 x 4-bit along N per word)
  scales:  [32, 4096] fp32, groupsize 128 (standard g_idx = k // 128)
  bias:    [4096] fp32

Sharding: 2-way on tokens x 4-way on out-features (core = ti*4 + ni).
Each core dequantizes its [4096, 1024] weight shard to bf16 in SBUF once
(int shift+mask on DVE/GPSIMD, scale multiply and zero-point subtract in
bf16), streams x tiles (cast fp32->bf16 in-DMA on the SWDGE ring,
transposed via the DMA xbar on the SP ring), and runs chunk-major
accumulating bf16 matmuls with fp32 PSUM over groups of 4 token tiles
(8 PSUM banks) so the TensorE keeps consuming W chunks while dequant
is still producing them.
"""

import numpy as np
from contextlib import ExitStack

import ml_dtypes
import concourse.bass as bass
import concourse.mybir as mybir
import concourse.tile as tile
from concourse import bacc
from concourse.bass_utils import run_bass_kernel_spmd

P = 128
GROUPSIZE = 128

# Full problem dims.
TOKENS_F, K_F, N_F = 8192, 4096, 4096
TSHARD, NSHARD = 2, 4
N_CORES = TSHARD * NSHARD
TGROUP = 1          # token tiles per emission group
PSUM_TILES = 8      # [128,512] fp32 psum tiles in flight (1 bank each)
GPSIMD_CHUNK_MOD = 1000  # gpsimd cannot do int32 shifts on trn2; keep dequant on DVE
DQ_BUFS = 2
XP_BUFS = 2
XTP_BUFS = 4
OP_BUFS = 4


def build_kernel(T_s=TOKENS_F // TSHARD, K=K_F, N_s=N_F // NSHARD,
                 no_dequant=False, no_xpath=False, no_matmul=False):
    """Build the per-core Bass program. T_s tokens, K contraction, N_s out dims."""
    assert T_s % P == 0 and K % P == 0 and N_s % 512 == 0
    C = K // P                 # contraction chunks == quant groups
    NB = N_s // 512            # 512-wide output column blocks
    TT = T_s // P              # token tiles

    nc = bacc.Bacc("TRN2", target_bir_lowering=False, debug=False)
    x_d = nc.dram_tensor("x", [T_s, K], mybir.dt.float32, kind="ExternalInput")
    qw_d = nc.dram_tensor("qw", [K // 8, N_s], mybir.dt.int32, kind="ExternalInput")
    # zs = (z+1)*scales precomputed host-side
    sc_d = nc.dram_tensor("sc", [C, N_s], mybir.dt.float32, kind="ExternalInput")
    zs_d = nc.dram_tensor("zs", [C, N_s], mybir.dt.float32, kind="ExternalInput")
    bias_d = nc.dram_tensor("bias", [N_s], mybir.dt.float32, kind="ExternalInput")
    out_d = nc.dram_tensor("out", [T_s, N_s], mybir.dt.float32, kind="ExternalOutput")

    shiftv_np = (4 * (np.arange(P) % 8)).astype(np.int32).reshape(P, 1)
    shiftv_d = nc.inline_tensor(shiftv_np, name="shiftv")

    with tile.TileContext(nc) as tc, ExitStack() as ctx:
        const = ctx.enter_context(tc.tile_pool(name="const", bufs=1))
        wpool = ctx.enter_context(tc.tile_pool(name="wpool", bufs=C))
        dq = ctx.enter_context(tc.tile_pool(name="dq", bufs=DQ_BUFS))
        xp = ctx.enter_context(tc.tile_pool(name="xp", bufs=XP_BUFS))
        xtp = ctx.enter_context(tc.tile_pool(name="xtp", bufs=XTP_BUFS))
        op = ctx.enter_context(tc.tile_pool(name="op", bufs=OP_BUFS))
        psum = ctx.enter_context(tc.tile_pool(name="psum", bufs=PSUM_TILES,
                                              space="PSUM"))

        # ---- constants ----
        shiftv0 = const.tile([P, 1], mybir.dt.int32)
        nc.scalar.dma_start(shiftv0[:], shiftv_d[:, :])
        shiftv = const.tile([P, 1], mybir.dt.int32)
        nc.vector.tensor_copy(shiftv[:], shiftv0[:])
        bias_rep0 = const.tile([P, N_s], mybir.dt.float32)
        nc.scalar.dma_start(bias_rep0[:], bass.AP(bias_d, 0, [[0, P], [1, N_s]]))
        bias_rep = const.tile([P, N_s], mybir.dt.float32)
        nc.vector.tensor_copy(bias_rep[:], bias_rep0[:])

        # ---- dequantize W chunk by chunk into resident bf16 [C][128, N_s] ----
        w_tiles = []
        for c in range(C):
            if no_dequant:
                w = wpool.tile([P, N_s], mybir.dt.bfloat16, tag="w")
                nc.gpsimd.memset(w[:], 0.25)
                w_tiles.append(w)
                continue
            eng = (nc.gpsimd if (not no_xpath) and
                   c % GPSIMD_CHUNK_MOD == GPSIMD_CHUNK_MOD - 1 else nc.vector)
            qb = dq.tile([P, N_s], mybir.dt.int32, tag="qb")
            # partition p = r*8 + j reads packed row 16c + r, all N_s cols
            src = bass.AP(qw_d, c * 16 * N_s, [[N_s, 16], [0, 8], [1, N_s]])
            nc.scalar.dma_start(qb[:], src)
            s_rep = dq.tile([P, N_s], mybir.dt.float32, tag="s_rep")
            nc.scalar.dma_start(s_rep[:], bass.AP(sc_d, c * N_s, [[0, P], [1, N_s]]))
            zs_rep = dq.tile([P, N_s], mybir.dt.float32, tag="zs_rep")
            nc.scalar.dma_start(zs_rep[:], bass.AP(zs_d, c * N_s, [[0, P], [1, N_s]]))

            # qb = (qb >> (4 * (p % 8))) & 0xF
            eng.tensor_tensor(
                qb[:], qb[:], shiftv[:].to_broadcast((P, N_s)),
                mybir.AluOpType.logical_shift_right,
            )
            eng.tensor_scalar(
                qb[:], qb[:], 0xF, None, mybir.AluOpType.bitwise_and,
            )
            # q4 = qb * s (int32 x f32 -> f32) ; W[c] = q4 - zs (bf16 out)
            q4 = dq.tile([P, N_s], mybir.dt.float32, tag="q4")
            eng.tensor_tensor(q4[:], qb[:], s_rep[:], mybir.AluOpType.mult)
            w = wpool.tile([P, N_s], mybir.dt.bfloat16, tag="w")
            eng.tensor_tensor(w[:], q4[:], zs_rep[:], mybir.AluOpType.subtract)
            w_tiles.append(w)

        # ---- x tiles: cast fp32->bf16 (SWDGE) + xbar transpose (SP ring) ----
        def make_xt(t):
            xt = xtp.tile([P, C, P], mybir.dt.bfloat16, tag="xt", name=f"xt{t}")
            if not no_xpath:
                x_bf = xp.tile([P, K], mybir.dt.bfloat16, tag="x_bf")
                nc.gpsimd.dma_start(x_bf[:], x_d[t * P:(t + 1) * P, :])
                nc.sync.dma_start_transpose(xt[:], x_bf[:])
            return xt

        # ---- matmuls: tile-major, scheduler interleaves across tiles ----
        for t in range(TT):
            xt = make_xt(t)
            psums = [psum.tile([P, 512], mybir.dt.float32, tag="ps",
                               name=f"ps{nb}") for nb in range(NB)]
            if no_matmul:
                for ps in psums:
                    nc.vector.tensor_copy(ps[:], xt[:, :4, :])
            else:
                for c in range(C):
                    lhsT = xt[:, c, :]
                    for nb in range(NB):
                        nc.tensor.matmul(
                            psums[nb][:], lhsT,
                            w_tiles[c][:, nb * 512:(nb + 1) * 512],
                            start=(c == 0), stop=(c == C - 1),
                        )
            for nb in range(NB):
                o = op.tile([P, 512], mybir.dt.float32, tag="o")
                nc.vector.tensor_tensor(
                    o[:], psums[nb][:], bias_rep[:, nb * 512:(nb + 1) * 512],
                    mybir.AluOpType.add,
                )
                nc.scalar.dma_start(
                    out_d[t * P:(t + 1) * P, nb * 512:(nb + 1) * 512], o[:],
                )

    nc.compile()
    return nc


_cache = {}


def _get_kernel(T_s, K, N_s):
    key = (T_s, K, N_s)
    if key not in _cache:
        _cache[key] = build_kernel(T_s, K, N_s)
    return _cache[key]


def make_in_maps(x, qweight, qzeros, scales, bias):
    """Split full inputs into per-core input dicts (2 token x 4 feature shards)."""
    t_sz = x.shape[0] // TSHARD
    n_sz = qweight.shape[1] // NSHARD
    # Unpack the (tiny) packed zero-points and fold the +1 and scale on host:
    # zs[g, n] = (z[g, n] + 1) * scales[g, n]; ship scales/zs as bf16.
    shifts = (np.arange(8, dtype=np.int32) * 4)
    z = ((qzeros[:, :, None] >> shifts[None, None, :]) & 0xF).reshape(
        qzeros.shape[0], -1)
    zs = ((z + 1).astype(np.float32) * scales).astype(np.float32)
    sc16 = scales
    in_maps = []
    for core in range(N_CORES):
        ti, ni = divmod(core, NSHARD)
        in_maps.append({
            "x": np.ascontiguousarray(x[ti * t_sz:(ti + 1) * t_sz, :]),
            "qw": np.ascontiguousarray(qweight[:, ni * n_sz:(ni + 1) * n_sz]),
            "sc": np.ascontiguousarray(sc16[:, ni * n_sz:(ni + 1) * n_sz]),
            "zs": np.ascontiguousarray(zs[:, ni * n_sz:(ni + 1) * n_sz]),
            "bias": np.ascontiguousarray(bias[ni * n_sz:(ni + 1) * n_sz]),
        })
    return in_maps


def assemble(results, tokens, n):
    t_sz = tokens // TSHARD
    n_sz = n // NSHARD
    out = np.empty((tokens, n), dtype=np.float32)
    for core in range(N_CORES):
        ti, ni = divmod(core, NSHARD)
        out[ti * t_sz:(ti + 1) * t_sz, ni * n_sz:(ni + 1) * n_sz] = results[core]["out"]
    return out


def kernel(x, qweight, qzeros, scales, g_idx, bias, _trace=False):
    x = np.asarray(x, dtype=np.float32)
    qweight = np.asarray(qweight, dtype=np.int32)
    qzeros = np.asarray(qzeros, dtype=np.int32)
    scales = np.asarray(scales, dtype=np.float32)
    bias = np.asarray(bias, dtype=np.float32)

    nc = _get_kernel(x.shape[0] // TSHARD, x.shape[1], qweight.shape[1] // NSHARD)
    in_maps = make_in_maps(x, qweight, qzeros, scales, bias)
    res = run_bass_kernel_spmd(
        nc, in_maps, core_ids=list(range(N_CORES)), trace=_trace,
    )
    out = assemble(res.results, x.shape[0], qweight.shape[1])
    if _trace:
        kernel.last_result = res
    return out
